# revision 22
# baseline (speedup 1.0000x reference)
"""Multi-head attention forward, sharded head-parallel across 8 NeuronCores.

Per core c (heads 2c, 2c+1), fp16 data path (fp8 adds ~3% error -- input
quantization noise scales WITH the signal through the contractions, it
does not average away):
  qT/kT/vT = (x @ W{q,k,v}_c.T).T       W.T-tiled matmuls vs fp16 xT,
                                        sequential q/k/v PSUM accumulation
  v1       = paired [128,128] PE transposes of vT (both heads at once),
             layout [kpos, KC, 2, (v_h|1)] with interleaved ones columns
  scoresT  = kT_chunk.T @ qT            [k-pos partitions, q-pos free],
             two heads on PE row-groups 0-63/64-127, diag blocks
             column-trimmed, ascending j so j=0 is full width
  probsT   = exp(scoresT) fp16, diag triangle zeroed by tri-mask mul
             (gpsimd, SBUF-only -- gpsimd may not touch PSUM)
  av+rowsum: pos = [v_h | 1].T @ probsT (ones column yields softmax denom)
  normalize: copy denom->SBUF, reciprocal (DVE), partition_broadcast
             (gpsimd), multiply pos x bs -> ocat (DVE)
  out_projT partial, casts distributed DVE/ACT, fp16 partials out
Host: sum the 8 partial [1024, 4096] fp16 outputs in fp32, transpose, bias.

Pitfall encoded here: custom DVE ops (reciprocal_approx_fast) read garbage
from PSUM on HW -- denominators are staged through SBUF first.
"""
import sys

sys.path.insert(0, "/opt/trn_rl_repo")

import ml_dtypes
import numpy as np

F16 = np.float16
F8 = ml_dtypes.float8_e4m3

B, S, D = 2, 2048, 1024
H, HD = 16, 64
NCORES = 8
SEC = 128           # output dims per core per section (2 heads * 64)
BS = B * S          # 4096
NT = BS // 512      # 8 seq tiles of 512
EC = D // 128       # 8 embed chunks
QT = S // 512       # 4 q-tiles per (b,h)
KC = S // 128       # 16 k-chunks per (b,h)

_cache = {}


def _build(mask_mode):
    import concourse.bass as bass
    import concourse.tile as tile
    from concourse import bacc, mybir

    f32 = mybir.dt.float32
    f16 = mybir.dt.float16
    f8 = mybir.dt.float8e4
    DR = mybir.MatmulPerfMode.DoubleRow
    Exp = mybir.ActivationFunctionType.Exp

    nc = bacc.Bacc("TRN2", target_bir_lowering=False, debug=False,
                   num_devices=NCORES)

    xT = nc.dram_tensor("xT", [D, BS], f16, kind="ExternalInput")
    wqkvT = nc.dram_tensor("wqkvT", [D, 3 * SEC], f16, kind="ExternalInput")
    woT = nc.dram_tensor("woT", [SEC, D], f16, kind="ExternalInput")
    # consts: [:, 0:128] = eye(128); [:, 128:384] = tri duplicated (2x128)
    consts = nc.dram_tensor("consts", [128, 384], f16, kind="ExternalInput")
    if mask_mode == "general":
        maskT = nc.dram_tensor("maskT", [S, S], f16, kind="ExternalInput")
    out_pT = nc.dram_tensor("out_pT", [D, BS], f16, kind="ExternalOutput")

    causal = mask_mode == "causal"

    with tile.TileContext(nc) as tc:
        with (
            nc.allow_low_precision(reason="fp16/fp8 attention pipeline"),
            tc.tile_pool(name="singles", bufs=1) as singles,
            tc.tile_pool(name="qkv", bufs=1) as qkv,
            tc.tile_pool(name="prp", bufs=4) as prp,
            tc.tile_pool(name="mskp", bufs=4) as mskp,
            tc.tile_pool(name="nrm", bufs=4) as nrm,
            tc.tile_pool(name="ftp", bufs=2) as ftp,
            tc.tile_pool(name="psA", bufs=2, space="PSUM") as psA,
            tc.tile_pool(name="psS", bufs=2, space="PSUM") as psS,
            tc.tile_pool(name="psO", bufs=2, space="PSUM") as psO,
        ):
            # ---- static loads (gpsimd queue) ----
            w_sb = singles.tile([128, EC, 3 * SEC], f16)
            wqr = wqkvT.rearrange("(ec p) c -> p ec c", p=128)
            for ec in range(EC):
                nc.gpsimd.dma_start(out=w_sb[:, ec, :], in_=wqr[:, ec, :])
            woT_sb = singles.tile([128, D], f16)
            nc.gpsimd.dma_start(out=woT_sb[:], in_=woT[:])
            ident = singles.tile([128, 128], f16)
            nc.gpsimd.dma_start(out=ident[:], in_=consts[:, 0:128])
            if causal:
                tri2 = singles.tile([128, 2, 128], f16)
                nc.gpsimd.dma_start(
                    out=tri2[:],
                    in_=consts[:, 128:384].rearrange("p (h c) -> p h c", h=2))

            # ---- x load, one DMA per 512-pos tile, 3 queues ----
            xfull = qkv.tile([128, EC, BS], f16)
            xTr = xT.rearrange("(ec p) s -> p ec s", p=128)
            xq = (nc.sync, nc.scalar, nc.gpsimd)
            for n in range(NT):
                sl = slice(512 * n, 512 * (n + 1))
                xq[n % 3].dma_start(out=xfull[:, :, sl], in_=xTr[:, :, sl])

            qT = qkv.tile([128, BS], f16)
            kT = qkv.tile([128, BS], f16)
            vT = qkv.tile([128, BS], f16)
            ocat = qkv.tile([128, BS], f16)
            v1s = []
            for b in range(B):
                v1 = qkv.tile([128, KC, 2, 65], f16, name=f"v1_{b}")
                v1s.append(v1)

            dsts = (qT, kT, vT)

            def sc_copy(out, in_):
                nc.scalar.copy(out, in_)

            def ve_copy(out, in_):
                nc.vector.tensor_copy(out, in_)

            # gpsimd must not touch PSUM (BIR verifier), so PSUM-reading
            # copies alternate DVE/ACT for b=0 (ACT idle before exp starts)
            # and go all-DVE for b=1 (ACT is exp-saturated then)
            def stage_a(ns):
                for n in ns:
                    nsl = slice(512 * n, 512 * (n + 1))
                    for part in range(3):
                        pa = psA.tile([128, 512], f32, tag="pa", name="pa")
                        csl = slice(128 * part, 128 * (part + 1))
                        for ec in range(EC):
                            nc.tensor.matmul(
                                pa[:], w_sb[:, ec, csl],
                                xfull[:, ec, nsl],
                                start=ec == 0, stop=ec == EC - 1)
                        nc.any.tensor_copy(dsts[part][:, nsl], pa[:])

            def stage_t(b, chunks):
                base = S * b
                v1 = v1s[b]
                for i in chunks:
                    pt = psA.tile([128, 128], f16, tag="pa", name="pt")
                    nc.tensor.transpose(
                        pt[:], vT[:, base + 128 * i:base + 128 * (i + 1)],
                        ident[:])
                    nc.any.tensor_copy(v1[:, i, :, 0:64],
                                       pt.rearrange("p (h c) -> p h c", h=2))

            def stage_b(b, ts, after_t=None):
                base = S * b
                v1 = v1s[b]
                for t in ts:
                    qsl = slice(base + 512 * t, base + 512 * (t + 1))
                    njc = 4 * t + 4 if causal else KC
                    pos = [psO.tile([65, 512], f32, tag="po",
                                    name=f"po{t}{lh}") for lh in range(2)]
                    for j in range(njc):
                        jm = j - 4 * t
                        c0 = 128 * jm if causal and jm >= 0 else 0
                        ksl = slice(base + 128 * j, base + 128 * (j + 1))
                        ps = psS.tile([128, 2, 512], f32, tag="ps", name="ps")
                        for lh in range(2):
                            hsl = slice(64 * lh, 64 * (lh + 1))
                            nc.tensor.matmul(
                                ps[:, lh, c0:], kT[hsl, ksl],
                                qT[hsl, qsl.start + c0:qsl.stop],
                                start=True, stop=True)
                        pr = prp.tile([128, 2, 512], f16, tag="pr", name="pr")
                        nc.scalar.activation(pr[:, :, c0:], ps[:, :, c0:],
                                             Exp)
                        if causal and jm >= 0:
                            win = slice(c0, c0 + 128)
                            nc.gpsimd.tensor_mul(pr[:, :, win],
                                                 pr[:, :, win], tri2[:])
                        elif mask_mode == "general":
                            msk = mskp.tile([128, 512], f16, tag="mk",
                                            name="msk")
                            nc.sync.dma_start(
                                out=msk[:],
                                in_=maskT[128 * j:128 * (j + 1),
                                          512 * t:512 * (t + 1)])
                            for lh in range(2):
                                nc.any.tensor_mul(pr[:, lh, :],
                                                  pr[:, lh, :], msk[:])
                        for lh in range(2):
                            nc.tensor.matmul(
                                pos[lh][:, c0:], v1[:, j, lh, :],
                                pr[:, lh, c0:],
                                start=j == 0, stop=j == njc - 1)
                    for lh in range(2):
                        hsl = slice(64 * lh, 64 * (lh + 1))
                        # custom DVE ops read garbage from PSUM on HW --
                        # stage the denominator row through SBUF first
                        lc = nrm.tile([1, 512], f32, tag="lc", name="lc")
                        nc.vector.tensor_copy(lc[:], pos[lh][64:65, :])
                        rc = nrm.tile([1, 512], f32, tag="rc", name="rc")
                        nc.vector.reciprocal_approx_fast(rc[:], lc[:])
                        bs_ = nrm.tile([64, 512], f32, tag="bs", name="bs")
                        nc.gpsimd.partition_broadcast(bs_[:], rc[:])
                        nc.any.tensor_mul(ocat[hsl, qsl],
                                          pos[lh][0:64, :], bs_[:])
                    if after_t is not None:
                        after_t(t)

            out_r = out_pT.rearrange("(oc p) s -> p oc s", p=128)

            def stage_c(ns):
                for n in ns:
                    ssl = slice(512 * n, 512 * (n + 1))
                    ft = ftp.tile([128, EC, 512], f16, tag="ft", name="ft")
                    for oc in range(EC):
                        osl = slice(128 * oc, 128 * (oc + 1))
                        pf = psA.tile([128, 512], f32, tag="pa", name="pf")
                        nc.tensor.matmul(pf[:], woT_sb[:, osl],
                                         ocat[:, ssl], start=True, stop=True)
                        nc.any.tensor_copy(ft[:, oc, :], pf[:])
                    nc.sync.dma_start(out=out_r[:, :, ssl], in_=ft[:])

            # emission order chosen so the psA tag ring (A/T/C share it)
            # never makes batch-1 stage A wait on batch-0 stage C
            nc.vector.memset(v1s[0][:, :, :, 64], 1.0)
            nc.vector.memset(v1s[1][:, :, :, 64], 1.0)
            for n in range(4):
                stage_a([n])
                stage_t(0, range(4 * n, 4 * n + 4))
                stage_b(0, [n])
            stage_a(range(4, 8))
            stage_t(1, range(KC))
            stage_c(range(0, 4))
            stage_b(1, range(QT), after_t=lambda t: stage_c([4 + t]))

    nc.compile()
    return nc


def _classify_mask(mask):
    m = np.asarray(mask).reshape(S, S) != 0
    if m.all():
        return "none", None
    if np.array_equal(m, np.tril(np.ones((S, S), bool))):
        return "causal", None
    return "general", m.T.astype(np.float32)


def _ensure_ntff_hook():
    """Register antenv.axon_hooks with a ctypes NTFF profile hook if the
    container image lacks it (mirrors trn_agent_boot's registration)."""
    import types
    try:
        from antenv.axon_hooks import get_axon_ntff_profile_hook  # noqa: F401
        return
    except ImportError:
        pass
    import contextlib
    import ctypes

    hook = None
    so_path = "/opt/axon/libaxon_pjrt.so"
    try:
        lib = ctypes.CDLL(so_path)
        if hasattr(lib, "axon_start_nrt_profile"):
            lib.axon_start_nrt_profile.argtypes = [
                ctypes.POINTER(ctypes.c_int64), ctypes.c_size_t]
            lib.axon_start_nrt_profile.restype = ctypes.c_int64
            lib.axon_stop_nrt_profile.argtypes = [ctypes.c_char_p]
            lib.axon_stop_nrt_profile.restype = ctypes.c_int64

            @contextlib.contextmanager
            def _hook(output_dir, device_ids):
                import jax
                jax.devices()
                if device_ids:
                    ids = (ctypes.c_int64 * len(device_ids))(*device_ids)
                    rc = lib.axon_start_nrt_profile(ids, len(device_ids))
                else:
                    rc = lib.axon_start_nrt_profile(None, 0)
                if rc != 0:
                    raise RuntimeError(f"axon_start_nrt_profile rc={rc}")
                try:
                    yield
                finally:
                    n = lib.axon_stop_nrt_profile(str(output_dir).encode())
                    print(f"profile: {n} file(s) written to {output_dir}",
                          flush=True)

            hook = _hook
    except OSError:
        pass

    mod = types.ModuleType("antenv.axon_hooks")
    _h = [hook]
    mod.get_axon_ntff_profile_hook = lambda: _h[0]

    def _set(h):
        _h[0] = h

    mod.set_axon_ntff_profile_hook = _set
    sys.modules["antenv.axon_hooks"] = mod
    try:
        import antenv
        antenv.axon_hooks = mod
    except ImportError:
        pass


def kernel(key, query, value, mask, W_qkv, W_out, b_out):
    from concourse.bass_utils import run_bass_kernel_spmd
    import os

    mask_mode, maskT = _classify_mask(mask)
    if mask_mode not in _cache:
        _cache[mask_mode] = _build(mask_mode)
    nc = _cache[mask_mode]

    x = np.ascontiguousarray(
        np.asarray(query, np.float32).reshape(BS, D))
    xT_f16 = np.ascontiguousarray(x.T).astype(F16)
    W_qkv = np.asarray(W_qkv, np.float32)
    W_out = np.asarray(W_out, np.float32)

    consts = np.zeros((128, 384), F16)
    consts[:, 0:128] = np.eye(128, dtype=F16)
    tri = (np.arange(128)[:, None] <= np.arange(128)[None, :]).astype(F16)
    consts[:, 128:256] = tri
    consts[:, 256:384] = tri

    in_maps = []
    for c in range(NCORES):
        sl = slice(SEC * c, SEC * (c + 1))
        wq = W_qkv[sl, :].T * np.float32(HD ** -0.5)
        wk = W_qkv[D + SEC * c:D + SEC * (c + 1), :].T
        wv = W_qkv[2 * D + SEC * c:2 * D + SEC * (c + 1), :].T
        m = {
            "xT": xT_f16,
            "consts": consts,
            "wqkvT": np.ascontiguousarray(np.concatenate(
                [wq, wk, wv], axis=1, dtype=np.float32)).astype(F16),
            "woT": np.ascontiguousarray(W_out[:, sl].T).astype(F16),
        }
        if mask_mode == "general":
            m["maskT"] = maskT.astype(F16)
        in_maps.append(m)

    trace = bool(int(os.environ.get("KERNEL_TRACE", "0")))
    if trace:
        _ensure_ntff_hook()
        try:
            res = run_bass_kernel_spmd(nc, in_maps,
                                       core_ids=list(range(NCORES)),
                                       trace=True)
        except Exception as e:
            print(f"traced run failed ({e!r}); retrying untraced",
                  flush=True)
            res = run_bass_kernel_spmd(nc, in_maps,
                                       core_ids=list(range(NCORES)))
        print(f"HW exec time: {res.exec_time_ns} ns", flush=True)
        kernel.last_exec_ns = res.exec_time_ns
        kernel.last_results = res
    else:
        res = run_bass_kernel_spmd(nc, in_maps, core_ids=list(range(NCORES)))
        kernel.last_results = res

    acc = res.results[0]["out_pT"].astype(np.float32)
    for c in range(1, NCORES):
        acc = acc + res.results[c]["out_pT"]
    out = acc.T.reshape(B, S, D) + np.asarray(b_out, np.float32)
    return out.astype(np.float32)


# revision 23
# speedup vs baseline: 1.5703x; 1.5703x over previous
"""Multi-head attention forward, sharded head-parallel across 8 NeuronCores.

Per core c (heads 2c, 2c+1), fp16 data path (fp8 adds ~3% error -- input
quantization noise scales WITH the signal through the contractions, it
does not average away):
  qT/kT/vT = (x @ W{q,k,v}_c.T).T       W.T-tiled matmuls vs fp16 xT,
                                        sequential q/k/v PSUM accumulation
  v1       = paired [128,128] PE transposes of vT (both heads at once),
             layout [kpos, KC, 2, (v_h|1)] with interleaved ones columns
  scoresT  = kT_chunk.T @ qT            [k-pos partitions, q-pos free],
             two heads on PE row-groups 0-63/64-127, diag blocks
             column-trimmed, ascending j so j=0 is full width
  probsT   = exp(scoresT) fp16, diag triangle zeroed by tri-mask mul
             (gpsimd, SBUF-only -- gpsimd may not touch PSUM)
  av+rowsum: pos = [v_h | 1].T @ probsT (ones column yields softmax denom)
  normalize: copy denom->SBUF, reciprocal (DVE), partition_broadcast
             (gpsimd), multiply pos x bs -> ocat (DVE)
  out_projT partial, casts distributed DVE/ACT, fp16 partials out
Host: sum the 8 partial [1024, 4096] fp16 outputs in fp32, transpose, bias.

Pitfall encoded here: custom DVE ops (reciprocal_approx_fast) read garbage
from PSUM on HW -- denominators are staged through SBUF first.
"""
import sys

sys.path.insert(0, "/opt/trn_rl_repo")

import ml_dtypes
import numpy as np

F16 = np.float16
F8 = ml_dtypes.float8_e4m3

B, S, D = 2, 2048, 1024
H, HD = 16, 64
NCORES = 8
SEC = 128           # output dims per core per section (2 heads * 64)
BS = B * S          # 4096
NT = BS // 512      # 8 seq tiles of 512
EC = D // 128       # 8 embed chunks
QT = S // 512       # 4 q-tiles per (b,h)
KC = S // 128       # 16 k-chunks per (b,h)

_cache = {}


def _build(mask_mode):
    import concourse.bass as bass
    import concourse.tile as tile
    from concourse import bacc, mybir

    f32 = mybir.dt.float32
    f16 = mybir.dt.float16
    f8 = mybir.dt.float8e4
    DR = mybir.MatmulPerfMode.DoubleRow
    Exp = mybir.ActivationFunctionType.Exp

    nc = bacc.Bacc("TRN2", target_bir_lowering=False, debug=False,
                   num_devices=NCORES)

    xT = nc.dram_tensor("xT", [D, BS], f16, kind="ExternalInput")
    wqkvT = nc.dram_tensor("wqkvT", [D, 3 * SEC], f16, kind="ExternalInput")
    woT = nc.dram_tensor("woT", [SEC, D], f16, kind="ExternalInput")
    # consts: [:, 0:128] = eye(128); [:, 128:384] = tri duplicated (2x128)
    consts = nc.dram_tensor("consts", [128, 384], f16, kind="ExternalInput")
    if mask_mode == "general":
        maskT = nc.dram_tensor("maskT", [S, S], f16, kind="ExternalInput")
    out_pT = nc.dram_tensor("out_pT", [D, BS], f16, kind="ExternalOutput")

    causal = mask_mode == "causal"

    with tile.TileContext(nc) as tc:
        with (
            nc.allow_low_precision(reason="fp16/fp8 attention pipeline"),
            tc.tile_pool(name="singles", bufs=1) as singles,
            tc.tile_pool(name="qkv", bufs=1) as qkv,
            tc.tile_pool(name="prp", bufs=4) as prp,
            tc.tile_pool(name="mskp", bufs=4) as mskp,
            tc.tile_pool(name="nrm", bufs=4) as nrm,
            tc.tile_pool(name="ftp", bufs=2) as ftp,
            tc.tile_pool(name="psA", bufs=2, space="PSUM") as psA,
            tc.tile_pool(name="psS", bufs=2, space="PSUM") as psS,
            tc.tile_pool(name="psO", bufs=2, space="PSUM") as psO,
        ):
            # ---- static loads (gpsimd queue) ----
            w_sb = singles.tile([128, EC, 3 * SEC], f16)
            wqr = wqkvT.rearrange("(ec p) c -> p ec c", p=128)
            for ec in range(EC):
                nc.gpsimd.dma_start(out=w_sb[:, ec, :], in_=wqr[:, ec, :])
            woT_sb = singles.tile([128, D], f16)
            nc.gpsimd.dma_start(out=woT_sb[:], in_=woT[:])
            ident = singles.tile([128, 128], f16)
            nc.gpsimd.dma_start(out=ident[:], in_=consts[:, 0:128])
            if causal:
                tri2 = singles.tile([128, 2, 128], f16)
                nc.gpsimd.dma_start(
                    out=tri2[:],
                    in_=consts[:, 128:384].rearrange("p (h c) -> p h c", h=2))

            # ---- x load, one DMA per 512-pos tile, 3 queues ----
            xfull = qkv.tile([128, EC, BS], f16)
            xTr = xT.rearrange("(ec p) s -> p ec s", p=128)
            for n in range(NT):
                sl = slice(512 * n, 512 * (n + 1))
                nc.sync.dma_start(out=xfull[:, :, sl], in_=xTr[:, :, sl])

            qT = qkv.tile([128, BS], f16)
            kT = qkv.tile([128, BS], f16)
            vT = qkv.tile([128, BS], f16)
            ocat = qkv.tile([128, BS], f16)
            v1s = []
            for b in range(B):
                v1 = qkv.tile([128, KC, 2, 65], f16, name=f"v1_{b}")
                v1s.append(v1)

            dsts = (qT, kT, vT)

            def sc_copy(out, in_):
                nc.scalar.copy(out, in_)

            def ve_copy(out, in_):
                nc.vector.tensor_copy(out, in_)

            # gpsimd must not touch PSUM (BIR verifier), so PSUM-reading
            # copies alternate DVE/ACT for b=0 (ACT idle before exp starts)
            # and go all-DVE for b=1 (ACT is exp-saturated then)
            def stage_a(ns):
                for n in ns:
                    nsl = slice(512 * n, 512 * (n + 1))
                    for part in range(3):
                        pa = psA.tile([128, 512], f32, tag="pa", name="pa")
                        csl = slice(128 * part, 128 * (part + 1))
                        for ec in range(EC):
                            nc.tensor.matmul(
                                pa[:], w_sb[:, ec, csl],
                                xfull[:, ec, nsl],
                                start=ec == 0, stop=ec == EC - 1)
                        nc.any.tensor_copy(dsts[part][:, nsl], pa[:])

            def stage_t(b, chunks):
                base = S * b
                v1 = v1s[b]
                nc.vector.memset(v1[:, :, :, 64], 1.0)
                for i in chunks:
                    pt = psA.tile([128, 128], f16, tag="pa", name="pt")
                    nc.tensor.transpose(
                        pt[:], vT[:, base + 128 * i:base + 128 * (i + 1)],
                        ident[:])
                    nc.any.tensor_copy(v1[:, i, :, 0:64],
                                       pt.rearrange("p (h c) -> p h c", h=2))

            def stage_b(b, ts, after_t=None):
                base = S * b
                v1 = v1s[b]
                for t in ts:
                    qsl = slice(base + 512 * t, base + 512 * (t + 1))
                    njc = 4 * t + 4 if causal else KC
                    pos = [psO.tile([65, 512], f32, tag="po",
                                    name=f"po{t}{lh}") for lh in range(2)]
                    for j in range(njc):
                        jm = j - 4 * t
                        c0 = 128 * jm if causal and jm >= 0 else 0
                        ksl = slice(base + 128 * j, base + 128 * (j + 1))
                        ps = psS.tile([128, 2, 512], f32, tag="ps", name="ps")
                        for lh in range(2):
                            hsl = slice(64 * lh, 64 * (lh + 1))
                            nc.tensor.matmul(
                                ps[:, lh, c0:], kT[hsl, ksl],
                                qT[hsl, qsl.start + c0:qsl.stop],
                                start=True, stop=True)
                        pr = prp.tile([128, 2, 512], f16, tag="pr", name="pr")
                        nc.scalar.activation(pr[:, :, c0:], ps[:, :, c0:],
                                             Exp)
                        if causal and jm >= 0:
                            win = slice(c0, c0 + 128)
                            nc.any.tensor_mul(pr[:, :, win],
                                              pr[:, :, win], tri2[:])
                        elif mask_mode == "general":
                            msk = mskp.tile([128, 512], f16, tag="mk",
                                            name="msk")
                            nc.sync.dma_start(
                                out=msk[:],
                                in_=maskT[128 * j:128 * (j + 1),
                                          512 * t:512 * (t + 1)])
                            for lh in range(2):
                                nc.any.tensor_mul(pr[:, lh, :],
                                                  pr[:, lh, :], msk[:])
                        for lh in range(2):
                            nc.tensor.matmul(
                                pos[lh][:, c0:], v1[:, j, lh, :],
                                pr[:, lh, c0:],
                                start=j == 0, stop=j == njc - 1)
                    for lh in range(2):
                        hsl = slice(64 * lh, 64 * (lh + 1))
                        # custom DVE ops read garbage from PSUM on HW --
                        # stage the denominator row through SBUF first
                        lc = nrm.tile([1, 512], f32, tag="lc", name="lc")
                        nc.vector.tensor_copy(lc[:], pos[lh][64:65, :])
                        rc = nrm.tile([1, 512], f32, tag="rc", name="rc")
                        nc.vector.reciprocal_approx_fast(rc[:], lc[:])
                        bs_ = nrm.tile([64, 512], f32, tag="bs", name="bs")
                        nc.gpsimd.partition_broadcast(bs_[:], rc[:])
                        nc.any.tensor_mul(ocat[hsl, qsl],
                                          pos[lh][0:64, :], bs_[:])
                    if after_t is not None:
                        after_t(t)

            out_r = out_pT.rearrange("(oc p) s -> p oc s", p=128)

            def stage_c(ns):
                for n in ns:
                    ssl = slice(512 * n, 512 * (n + 1))
                    ft = ftp.tile([128, EC, 512], f16, tag="ft", name="ft")
                    for oc in range(EC):
                        osl = slice(128 * oc, 128 * (oc + 1))
                        pf = psA.tile([128, 512], f32, tag="pa", name="pf")
                        nc.tensor.matmul(pf[:], woT_sb[:, osl],
                                         ocat[:, ssl], start=True, stop=True)
                        nc.any.tensor_copy(ft[:, oc, :], pf[:])
                    nc.sync.dma_start(out=out_r[:, :, ssl], in_=ft[:])

            # emission order chosen so the psA tag ring (A/T/C share it)
            # never makes batch-1 stage A wait on batch-0 stage C
            stage_a(range(0, 4))
            stage_t(0, range(KC))
            stage_b(0, range(QT))
            stage_a(range(4, 8))
            stage_t(1, range(KC))
            stage_c(range(0, 4))
            stage_b(1, range(QT))
            stage_c(range(4, 8))

    nc.compile()
    return nc


def _classify_mask(mask):
    m = np.asarray(mask).reshape(S, S) != 0
    if m.all():
        return "none", None
    if np.array_equal(m, np.tril(np.ones((S, S), bool))):
        return "causal", None
    return "general", m.T.astype(np.float32)


def _ensure_ntff_hook():
    """Register antenv.axon_hooks with a ctypes NTFF profile hook if the
    container image lacks it (mirrors trn_agent_boot's registration)."""
    import types
    try:
        from antenv.axon_hooks import get_axon_ntff_profile_hook  # noqa: F401
        return
    except ImportError:
        pass
    import contextlib
    import ctypes

    hook = None
    so_path = "/opt/axon/libaxon_pjrt.so"
    try:
        lib = ctypes.CDLL(so_path)
        if hasattr(lib, "axon_start_nrt_profile"):
            lib.axon_start_nrt_profile.argtypes = [
                ctypes.POINTER(ctypes.c_int64), ctypes.c_size_t]
            lib.axon_start_nrt_profile.restype = ctypes.c_int64
            lib.axon_stop_nrt_profile.argtypes = [ctypes.c_char_p]
            lib.axon_stop_nrt_profile.restype = ctypes.c_int64

            @contextlib.contextmanager
            def _hook(output_dir, device_ids):
                import jax
                jax.devices()
                if device_ids:
                    ids = (ctypes.c_int64 * len(device_ids))(*device_ids)
                    rc = lib.axon_start_nrt_profile(ids, len(device_ids))
                else:
                    rc = lib.axon_start_nrt_profile(None, 0)
                if rc != 0:
                    raise RuntimeError(f"axon_start_nrt_profile rc={rc}")
                try:
                    yield
                finally:
                    n = lib.axon_stop_nrt_profile(str(output_dir).encode())
                    print(f"profile: {n} file(s) written to {output_dir}",
                          flush=True)

            hook = _hook
    except OSError:
        pass

    mod = types.ModuleType("antenv.axon_hooks")
    _h = [hook]
    mod.get_axon_ntff_profile_hook = lambda: _h[0]

    def _set(h):
        _h[0] = h

    mod.set_axon_ntff_profile_hook = _set
    sys.modules["antenv.axon_hooks"] = mod
    try:
        import antenv
        antenv.axon_hooks = mod
    except ImportError:
        pass


def kernel(key, query, value, mask, W_qkv, W_out, b_out):
    from concourse.bass_utils import run_bass_kernel_spmd
    import os

    mask_mode, maskT = _classify_mask(mask)
    if mask_mode not in _cache:
        _cache[mask_mode] = _build(mask_mode)
    nc = _cache[mask_mode]

    x = np.ascontiguousarray(
        np.asarray(query, np.float32).reshape(BS, D))
    xT_f16 = np.ascontiguousarray(x.T).astype(F16)
    W_qkv = np.asarray(W_qkv, np.float32)
    W_out = np.asarray(W_out, np.float32)

    consts = np.zeros((128, 384), F16)
    consts[:, 0:128] = np.eye(128, dtype=F16)
    tri = (np.arange(128)[:, None] <= np.arange(128)[None, :]).astype(F16)
    consts[:, 128:256] = tri
    consts[:, 256:384] = tri

    in_maps = []
    for c in range(NCORES):
        sl = slice(SEC * c, SEC * (c + 1))
        wq = W_qkv[sl, :].T * np.float32(HD ** -0.5)
        wk = W_qkv[D + SEC * c:D + SEC * (c + 1), :].T
        wv = W_qkv[2 * D + SEC * c:2 * D + SEC * (c + 1), :].T
        m = {
            "xT": xT_f16,
            "consts": consts,
            "wqkvT": np.ascontiguousarray(np.concatenate(
                [wq, wk, wv], axis=1, dtype=np.float32)).astype(F16),
            "woT": np.ascontiguousarray(W_out[:, sl].T).astype(F16),
        }
        if mask_mode == "general":
            m["maskT"] = maskT.astype(F16)
        in_maps.append(m)

    trace = bool(int(os.environ.get("KERNEL_TRACE", "0")))
    if trace:
        _ensure_ntff_hook()
        try:
            res = run_bass_kernel_spmd(nc, in_maps,
                                       core_ids=list(range(NCORES)),
                                       trace=True)
        except Exception as e:
            print(f"traced run failed ({e!r}); retrying untraced",
                  flush=True)
            res = run_bass_kernel_spmd(nc, in_maps,
                                       core_ids=list(range(NCORES)))
        print(f"HW exec time: {res.exec_time_ns} ns", flush=True)
        kernel.last_exec_ns = res.exec_time_ns
        kernel.last_results = res
    else:
        res = run_bass_kernel_spmd(nc, in_maps, core_ids=list(range(NCORES)))
        kernel.last_results = res

    acc = res.results[0]["out_pT"].astype(np.float32)
    for c in range(1, NCORES):
        acc = acc + res.results[c]["out_pT"]
    out = acc.T.reshape(B, S, D) + np.asarray(b_out, np.float32)
    return out.astype(np.float32)
